# revision 25
# baseline (speedup 1.0000x reference)
"""Windowed sparse attention kernel for TRN2 (8 NeuronCores).

Problem: b=1, h=16, n=16384, d=32, window w=128, nw=128 windows.
Each window of 128 queries attends to [4 memory slots | prev window | cur window]
with additive bias, tanh softcap (50), softmax.

Sharding: sequence-parallel over windows. Core c handles windows
[c*16, (c+1)*16) for all 16 heads, with a one-window k/v halo.

Device dataflow (keys on partitions, slot-major, bf16 matmuls):
  For k/v slot s (17 per core), one N=256 bf16 matmul computes
  simT[key_s, (q_{s-1} | q_s)] — slot s's keys against both query windows
  that attend to it. Heads are processed in pairs on distinct 32-row
  PE strips (tile_position) so their K=32 matmuls overlap in the array.
  DVE adds the (pre-arranged, mask-folded) bf16 bias while evacuating
  PSUM in 1024-col chunks into a pair-wide fp32 sim tile. ACT applies
  tanh (in-place) + exp in two 8448-col instructions per pair (exp
  output bf16), skipping the filler columns at both ends. mm2 flips
  orientation: lhsT = p-slice (keys x queries, bf16), rhs = v~
  (keys x 33, bf16) -> out (128 q, 33) per task, where v~'s ones column
  makes col 32 the softmax denominator Z. Host combines the 4-slot
  memory attention (1.5% of keys) and normalizes.
"""

import numpy as np
import ml_dtypes

B, H, N, D = 1, 16, 16384, 32
W = 128                 # window size
NW = N // W             # 128 windows
NCORES = 8
WPC = NW // NCORES      # 16 windows (tasks) per core
NSLOT = WPC + 1         # 17 k/v slots (halo)
SOFTCLAMP = 50.0
SCALE = D ** -0.5
MASK_PEN = -30000.0
SIMW = NSLOT * 256      # 4352 wide-tile cols (slot-major, 256 per slot)
CHUNK = 1536            # PSUM evacuation chunk (3 banks, 6 slots)

BF16 = ml_dtypes.bfloat16

_COMPILED = None


def _build_bass():
    import concourse.bacc as bacc
    import concourse.tile as tile
    from concourse import mybir
    from contextlib import ExitStack

    f32 = mybir.dt.float32
    bf16 = mybir.dt.bfloat16
    nc = bacc.Bacc()

    qT = nc.declare_dram_parameter("qT", [4, 128, WPC * W], bf16, isOutput=False)
    kT = nc.declare_dram_parameter("kT", [4, 128, NSLOT * W], bf16, isOutput=False)
    vv = nc.declare_dram_parameter("vv", [H, 128, NSLOT * 33], bf16, isOutput=False)
    bT = nc.declare_dram_parameter("bT", [128, SIMW], bf16, isOutput=False)
    o = nc.declare_dram_parameter("o", [H, 128, WPC * 33], bf16, isOutput=True)

    # chunk layout: (col_start, ncols, slots)
    chunks = [(0, 1536, list(range(0, 6))),
              (1536, 1536, list(range(6, 12))),
              (3072, 1280, list(range(12, 17)))]

    with ExitStack() as ctx:
        tc = ctx.enter_context(tile.TileContext(nc))
        singles = ctx.enter_context(tc.tile_pool(name="singles", bufs=1))
        qk_pool = ctx.enter_context(tc.tile_pool(name="qk", bufs=2))
        v_pool = ctx.enter_context(tc.tile_pool(name="v", bufs=4))
        sim_pool = ctx.enter_context(tc.tile_pool(name="sims", bufs=3))
        p_pool = ctx.enter_context(tc.tile_pool(name="ps", bufs=2))
        ow_pool = ctx.enter_context(tc.tile_pool(name="ow", bufs=2))
        sim_ps = ctx.enter_context(tc.tile_pool(name="simps", bufs=2, space="PSUM"))
        out_ps = ctx.enter_context(tc.tile_pool(name="outps", bufs=2, space="PSUM"))

        # per-chunk bias tiles: the first DVE add only waits on its own slice
        biasC = [None, None, None]

        def emit_bias(ci):
            cs, csz, _ = chunks[ci]
            bt = singles.tile([128, csz], bf16, name=f"bias{ci}")
            eng = nc.gpsimd if ci == 0 else nc.sync
            eng.dma_start(out=bt[:, :], in_=bT[:, cs:cs + csz])
            biasC[ci] = bt

        # K/Q tiles per PSUM chunk: chunk ci's matmuls gate only on slice ci.
        # K slices by slot: [0:6), [6:12), [12:17); Q slices by window with
        # one-window overlap: [0:6), [5:12), [11:16).
        KSL = [(0, 6 * W), (6 * W, 12 * W), (12 * W, NSLOT * W)]
        QSL = [(0, 6 * W), (5 * W, 12 * W), (11 * W, WPC * W)]
        for g in range(4):
            Ks, Qs = [], []
            for ci in range(3):
                # the idle GpSimd engine's preamble ends earliest — its SWDGE
                # issues the critical first transfers ~2 µs sooner than HWDGE
                dma = nc.gpsimd if (g == 0 and ci == 0) else nc.sync
                ks0, ks1 = KSL[ci]
                Kc = qk_pool.tile([128, ks1 - ks0], bf16, tag=f"k{ci}",
                                  name=f"k{ci}g{g}")
                dma.dma_start(out=Kc[:, :], in_=kT[g][:, ks0:ks1])
                qs0, qs1 = QSL[ci]
                Qc = qk_pool.tile([128, qs1 - qs0], bf16, tag=f"q{ci}",
                                  name=f"q{ci}g{g}")
                dma.dma_start(out=Qc[:, :], in_=qT[g][:, qs0:qs1])
                Ks.append(Kc)
                Qs.append(Qc)
                if g == 0:
                    emit_bias(ci)       # need-order: K_ci, Q_ci, bias_ci

            def k_ap(ci, p0, s):
                lo = s * W - KSL[ci][0]
                return Ks[ci][p0:p0 + 32, lo:lo + W]

            def q_ap(ci, p0, w, nw):    # query windows w .. w+nw-1
                lo = w * W - QSL[ci][0]
                return Qs[ci][p0:p0 + 32, lo:lo + nw * W]

            for j in range(2):      # head pairs within group, strips (64j, 64j+32)
                pj = 2 * g + j      # pair index 0..7
                pair = [(4 * g + 2 * j + u, 32 * (2 * j + u)) for u in range(2)]
                simS = sim_pool.tile([128, 2 * SIMW], f32, tag="simS")

                def emit_mm1(hs):
                    # mm1 + bias-add for the heads in hs, chunk by chunk,
                    # interleaved across their row strips
                    for ci, (cs, csz, slots) in enumerate(chunks):
                        simPs = {h: sim_ps.tile([128, CHUNK], f32, tag="simP",
                                                name=f"simP{h}c{ci}")
                                 for h, _ in hs}
                        for s in slots:
                            off = s * 256 - cs
                            for h, p0 in hs:
                                simP = simPs[h]
                                lhsT = k_ap(ci, p0, s)
                                if s == 0:
                                    # cols [0:128] (task -1) are filler: never
                                    # computed, skipped by ACT, unread by mm2
                                    nc.tensor.matmul(simP[:, 128:256], lhsT=lhsT,
                                                     rhs=q_ap(ci, p0, 0, 1),
                                                     start=True, stop=True,
                                                     tile_position=(p0, 0))
                                elif s == NSLOT - 1:
                                    nc.tensor.matmul(simP[:, off:off + 128], lhsT=lhsT,
                                                     rhs=q_ap(ci, p0, s - 1, 1),
                                                     start=True, stop=True,
                                                     tile_position=(p0, 0))
                                else:
                                    nc.tensor.matmul(simP[:, off:off + 256], lhsT=lhsT,
                                                     rhs=q_ap(ci, p0, s - 1, 2),
                                                     start=True, stop=True,
                                                     tile_position=(p0, 0))
                        for h, _ in hs:
                            u = 0 if h == pair[0][0] else 1
                            nc.vector.tensor_add(
                                simS[:, u * SIMW + cs:u * SIMW + cs + csz],
                                simPs[h][:, 0:csz],
                                biasC[ci][:, 0:csz],
                            )

                if pj == 0:
                    # head-serial at the very start: h0's sim completes (and
                    # ACT starts) ~3 µs earlier
                    emit_mm1([pair[0]])
                    emit_mm1([pair[1]])
                else:
                    emit_mm1(pair)
                # V DMAs issued after the K/Q/bias stream (mm2 needs them
                # only ~15 µs later; keeps them off the critical early DMAs)
                Vhs = {}
                for h, p0 in pair:
                    Vh = v_pool.tile([128, NSLOT * 33], bf16, tag="vh",
                                     name=f"vh{h}")
                    nc.sync.dma_start(out=Vh[:, :], in_=vv[h])
                    Vhs[h] = Vh
                # softcap + exp, skipping the 128 filler cols at each end
                # (contents may be garbage). First/last pairs run per-head
                # (last head per-half) ops to shorten the pipeline
                # fill/drain; middle pairs run one wide op per pass.
                pS = p_pool.tile([128, 2 * SIMW], bf16, tag="pS")
                if pj == 0:
                    spans = [(128, SIMW - 128), (SIMW + 128, 2 * SIMW - 128)]
                elif pj == 7:
                    spans = [(128, SIMW - 128),
                             (SIMW + 128, SIMW + 2304),
                             (SIMW + 2304, 2 * SIMW - 128)]
                else:
                    spans = [(128, 2 * SIMW - 128)]
                if pj == 0:
                    # tanh both heads first: the second tanh hides the
                    # same-tile write->read turnaround before the first exp
                    for lo, hi in spans:
                        nc.scalar.activation(simS[:, lo:hi], simS[:, lo:hi],
                                             mybir.ActivationFunctionType.Tanh,
                                             scale=1.0 / SOFTCLAMP)
                    for lo, hi in spans:
                        nc.scalar.activation(pS[:, lo:hi], simS[:, lo:hi],
                                             mybir.ActivationFunctionType.Exp,
                                             scale=SOFTCLAMP)
                else:
                    for lo, hi in spans:
                        nc.scalar.activation(simS[:, lo:hi], simS[:, lo:hi],
                                             mybir.ActivationFunctionType.Tanh,
                                             scale=1.0 / SOFTCLAMP)
                        nc.scalar.activation(pS[:, lo:hi], simS[:, lo:hi],
                                             mybir.ActivationFunctionType.Exp,
                                             scale=SOFTCLAMP)
                # mm2: out (128 q, 33) per task, 8 tasks per PSUM bank
                for u, (h, _) in enumerate(pair):
                    Vh = Vhs[h]
                    base = u * SIMW
                    outW = ow_pool.tile([128, WPC * 33], bf16, tag="outW",
                                        name=f"outW{h}")
                    for tb in range(2):
                        otP = out_ps.tile([128, 8 * 33], f32, tag="otP",
                                          name=f"otP{h}_{tb}")
                        for uu in range(8):
                            t = 8 * tb + uu
                            # prev: slot t keys, q_t = second half of slot t block
                            nc.tensor.matmul(
                                otP[:, uu * 33:(uu + 1) * 33],
                                lhsT=pS[:, base + t * 256 + 128:base + t * 256 + 256],
                                rhs=Vh[:, t * 33:(t + 1) * 33],
                                start=True, stop=False)
                            # cur: slot t+1 keys, q_t = first half of slot t+1 block
                            nc.tensor.matmul(
                                otP[:, uu * 33:(uu + 1) * 33],
                                lhsT=pS[:, base + (t + 1) * 256:base + (t + 1) * 256 + 128],
                                rhs=Vh[:, (t + 1) * 33:(t + 2) * 33],
                                start=False, stop=True)
                        nc.vector.tensor_copy(outW[:, tb * 264:(tb + 1) * 264], otP[:, :])
                        nc.sync.dma_start(out=o[h][:, tb * 264:(tb + 1) * 264],
                                          in_=outW[:, tb * 264:(tb + 1) * 264])
    nc.compile()
    return nc


def _get_compiled():
    global _COMPILED
    if _COMPILED is None:
        _COMPILED = _build_bass()
    return _COMPILED


def _prep_core(c, qs, ks, vs, ab, mvec):
    """Build per-core input arrays. qs,ks,vs: (H, N, D) (qs pre-scaled)."""
    w0 = c * WPC
    qw = qs.reshape(H, NW, W, D)[:, w0:w0 + WPC]          # (H,16,128,32)
    qTc = np.ascontiguousarray(
        qw.reshape(4, 4, WPC, W, D).transpose(0, 1, 4, 2, 3).reshape(4, 128, WPC * W))

    kw = ks.reshape(H, NW, W, D)
    vw = vs.reshape(H, NW, W, D)
    khalo = np.zeros((H, NSLOT, W, D), np.float32)
    vhalo = np.zeros((H, NSLOT, W, D), np.float32)
    lo = w0 - 1
    src_lo = max(lo, 0)
    dst_lo = src_lo - lo
    khalo[:, dst_lo:] = kw[:, src_lo:w0 + WPC]
    vhalo[:, dst_lo:] = vw[:, src_lo:w0 + WPC]
    kTc = np.ascontiguousarray(
        khalo.reshape(4, 4, NSLOT, W, D).transpose(0, 1, 4, 2, 3).reshape(4, 128, NSLOT * W))
    vvc = np.concatenate([vhalo, np.ones((H, NSLOT, W, 1), np.float32)], axis=3)
    vvc = np.ascontiguousarray(
        vvc.transpose(0, 2, 1, 3).reshape(H, 128, NSLOT * 33))

    # bias, slot-major: slot s block cols = [cur-bias(task s-1) | prev-bias(task s)]
    # both halves use keys of global window w0+s-1; fold key mask (+ structural
    # masking of window -1) as additive penalty.
    bTc = np.zeros((128, NSLOT, 2, W), np.float32)         # (key, slot, half, q)
    for s in range(NSLOT):
        gw = w0 + s - 1
        if s > 0:
            bTc[:, s, 0, :] = ab[gw, :, 128:256].T          # cur role for task s-1
        if s < NSLOT - 1:
            bTc[:, s, 1, :] = ab[gw + 1, :, 0:128].T        # prev role for task s
        if gw < 0:
            pen = np.full((W,), MASK_PEN, np.float32)
        else:
            pen = np.where(mvec[gw * W:(gw + 1) * W], np.float32(0),
                           np.float32(MASK_PEN))
        bTc[:, s, :, :] += pen[:, None, None]
    bTc = np.ascontiguousarray(bTc.reshape(128, SIMW))
    return {"qT": qTc.astype(BF16), "kT": kTc.astype(BF16),
            "vv": vvc.astype(BF16), "bT": bTc.astype(BF16)}


def _run_device(in_maps, trace=False):
    from concourse.bass_utils import run_bass_kernel_spmd
    nc = _get_compiled()
    res = run_bass_kernel_spmd(nc, in_maps, list(range(NCORES)), trace=trace)
    return res


def kernel(q, k, v, mask, attn_bias, memory_kv, _trace=False, _ret_res=False):
    q = np.asarray(q, np.float32)
    k = np.asarray(k, np.float32)
    v = np.asarray(v, np.float32)
    mask = np.asarray(mask)
    attn_bias = np.asarray(attn_bias, np.float32)
    memory_kv = np.asarray(memory_kv, np.float32)

    qs = q[0] * np.float32(SCALE)       # (H, N, D)
    ks, vs = k[0], v[0]
    ab = attn_bias[0]                   # (NW, W, 2W)
    mvec = mask[0].astype(bool)         # (N,)

    in_maps = [_prep_core(c, qs, ks, vs, ab, mvec) for c in range(NCORES)]
    res = _run_device(in_maps, trace=_trace)
    outs = [r["o"] for r in res.results]             # each (H, 128, WPC*33)

    big = np.stack(outs)                              # (8, H, 128, 528)
    # (core, h, q, task, 33) -> (h, core, task, q, 33) -> (h, n, 33)
    arr = big.reshape(NCORES, H, W, WPC, 33).transpose(1, 0, 3, 2, 4)
    arr = arr.reshape(H, N, 33)
    num = arr[..., :D].astype(np.float64)             # (H, N, D)
    z = arr[..., D].astype(np.float64)                # (H, N)

    # memory-slot attention (4 keys, no bias, mask=True) on host
    mk, mv = memory_kv[0], memory_kv[1]               # (H, 4, D)
    sim_m = np.einsum('hnd,hmd->hnm', qs, mk, dtype=np.float64)
    pm = np.exp(SOFTCLAMP * np.tanh(sim_m / SOFTCLAMP))
    num = num + np.einsum('hnm,hmd->hnd', pm, mv.astype(np.float64))
    z = z + pm.sum(-1)

    out = (num / z[..., None]).astype(np.float32)[None]   # (1, H, N, D)
    if _ret_res:
        return out, res
    return out


# revision 27
# speedup vs baseline: 1.0011x; 1.0011x over previous
"""Windowed sparse attention kernel for TRN2 (8 NeuronCores).

Problem: b=1, h=16, n=16384, d=32, window w=128, nw=128 windows.
Each window of 128 queries attends to [4 memory slots | prev window | cur window]
with additive bias, tanh softcap (50), softmax.

Sharding: sequence-parallel over windows. Core c handles windows
[c*16, (c+1)*16) for all 16 heads, with a one-window k/v halo.

Device dataflow (keys on partitions, slot-major, bf16 matmuls):
  For k/v slot s (17 per core), one N=256 bf16 matmul computes
  simT[key_s, (q_{s-1} | q_s)] — slot s's keys against both query windows
  that attend to it. Heads are processed in pairs on distinct 32-row
  PE strips (tile_position) so their K=32 matmuls overlap in the array.
  DVE adds the (pre-arranged, mask-folded) bf16 bias while evacuating
  PSUM in 1024-col chunks into a pair-wide fp32 sim tile. ACT applies
  tanh (in-place) + exp in two 8448-col instructions per pair (exp
  output bf16), skipping the filler columns at both ends. mm2 flips
  orientation: lhsT = p-slice (keys x queries, bf16), rhs = v~
  (keys x 33, bf16) -> out (128 q, 33) per task, where v~'s ones column
  makes col 32 the softmax denominator Z. Host combines the 4-slot
  memory attention (1.5% of keys) and normalizes.
"""

import numpy as np
import ml_dtypes

B, H, N, D = 1, 16, 16384, 32
W = 128                 # window size
NW = N // W             # 128 windows
NCORES = 8
WPC = NW // NCORES      # 16 windows (tasks) per core
NSLOT = WPC + 1         # 17 k/v slots (halo)
SOFTCLAMP = 50.0
SCALE = D ** -0.5
MASK_PEN = -30000.0
SIMW = NSLOT * 256      # 4352 wide-tile cols (slot-major, 256 per slot)
CHUNK = 1536            # PSUM evacuation chunk (3 banks, 6 slots)

BF16 = ml_dtypes.bfloat16

_COMPILED = None


def _build_bass():
    import concourse.bacc as bacc
    import concourse.tile as tile
    from concourse import mybir
    from contextlib import ExitStack

    f32 = mybir.dt.float32
    bf16 = mybir.dt.bfloat16
    nc = bacc.Bacc()

    qT = nc.declare_dram_parameter("qT", [4, 128, WPC * W], bf16, isOutput=False)
    kT = nc.declare_dram_parameter("kT", [4, 128, NSLOT * W], bf16, isOutput=False)
    vv = nc.declare_dram_parameter("vv", [H, 128, NSLOT * 33], bf16, isOutput=False)
    bT = nc.declare_dram_parameter("bT", [128, SIMW], bf16, isOutput=False)
    o = nc.declare_dram_parameter("o", [H, 128, WPC * 33], f32, isOutput=True)

    # chunk layout: (col_start, ncols, slots)
    chunks = [(0, 1536, list(range(0, 6))),
              (1536, 1536, list(range(6, 12))),
              (3072, 1280, list(range(12, 17)))]

    with ExitStack() as ctx:
        tc = ctx.enter_context(tile.TileContext(nc))
        singles = ctx.enter_context(tc.tile_pool(name="singles", bufs=1))
        qk_pool = ctx.enter_context(tc.tile_pool(name="qk", bufs=2))
        v_pool = ctx.enter_context(tc.tile_pool(name="v", bufs=4))
        sim_pool = ctx.enter_context(tc.tile_pool(name="sims", bufs=3))
        p_pool = ctx.enter_context(tc.tile_pool(name="ps", bufs=2))
        ow_pool = ctx.enter_context(tc.tile_pool(name="ow", bufs=2))
        sim_ps = ctx.enter_context(tc.tile_pool(name="simps", bufs=2, space="PSUM"))
        out_ps = ctx.enter_context(tc.tile_pool(name="outps", bufs=2, space="PSUM"))

        # per-chunk bias tiles: the first DVE add only waits on its own slice
        biasC = [None, None, None]

        def emit_bias(ci):
            cs, csz, _ = chunks[ci]
            bt = singles.tile([128, csz], bf16, name=f"bias{ci}")
            eng = nc.gpsimd if ci == 0 else nc.sync
            eng.dma_start(out=bt[:, :], in_=bT[:, cs:cs + csz])
            biasC[ci] = bt

        # K/Q tiles per PSUM chunk: chunk ci's matmuls gate only on slice ci.
        # K slices by slot: [0:6), [6:12), [12:17); Q slices by window with
        # one-window overlap: [0:6), [5:12), [11:16).
        KSL = [(0, 6 * W), (6 * W, 12 * W), (12 * W, NSLOT * W)]
        QSL = [(0, 6 * W), (5 * W, 12 * W), (11 * W, WPC * W)]
        for g in range(4):
            Ks, Qs = [], []
            for ci in range(3):
                # the idle GpSimd engine's preamble ends earliest — its SWDGE
                # issues the critical first transfers ~2 µs sooner than HWDGE
                dma = nc.gpsimd if (g == 0 and ci == 0) else nc.sync
                ks0, ks1 = KSL[ci]
                Kc = qk_pool.tile([128, ks1 - ks0], bf16, tag=f"k{ci}",
                                  name=f"k{ci}g{g}")
                dma.dma_start(out=Kc[:, :], in_=kT[g][:, ks0:ks1])
                qs0, qs1 = QSL[ci]
                Qc = qk_pool.tile([128, qs1 - qs0], bf16, tag=f"q{ci}",
                                  name=f"q{ci}g{g}")
                dma.dma_start(out=Qc[:, :], in_=qT[g][:, qs0:qs1])
                Ks.append(Kc)
                Qs.append(Qc)
                if g == 0:
                    emit_bias(ci)       # need-order: K_ci, Q_ci, bias_ci

            def k_ap(ci, p0, s):
                lo = s * W - KSL[ci][0]
                return Ks[ci][p0:p0 + 32, lo:lo + W]

            def q_ap(ci, p0, w, nw):    # query windows w .. w+nw-1
                lo = w * W - QSL[ci][0]
                return Qs[ci][p0:p0 + 32, lo:lo + nw * W]

            for j in range(2):      # head pairs within group, strips (64j, 64j+32)
                pj = 2 * g + j      # pair index 0..7
                pair = [(4 * g + 2 * j + u, 32 * (2 * j + u)) for u in range(2)]
                simS = sim_pool.tile([128, 2 * SIMW], f32, tag="simS")

                def emit_mm1(hs):
                    # mm1 + bias-add for the heads in hs, chunk by chunk,
                    # interleaved across their row strips
                    for ci, (cs, csz, slots) in enumerate(chunks):
                        simPs = {h: sim_ps.tile([128, CHUNK], f32, tag="simP",
                                                name=f"simP{h}c{ci}")
                                 for h, _ in hs}
                        for s in slots:
                            off = s * 256 - cs
                            for h, p0 in hs:
                                simP = simPs[h]
                                lhsT = k_ap(ci, p0, s)
                                if s == 0:
                                    # cols [0:128] (task -1) are filler: never
                                    # computed, skipped by ACT, unread by mm2
                                    nc.tensor.matmul(simP[:, 128:256], lhsT=lhsT,
                                                     rhs=q_ap(ci, p0, 0, 1),
                                                     start=True, stop=True,
                                                     tile_position=(p0, 0))
                                elif s == NSLOT - 1:
                                    nc.tensor.matmul(simP[:, off:off + 128], lhsT=lhsT,
                                                     rhs=q_ap(ci, p0, s - 1, 1),
                                                     start=True, stop=True,
                                                     tile_position=(p0, 0))
                                else:
                                    nc.tensor.matmul(simP[:, off:off + 256], lhsT=lhsT,
                                                     rhs=q_ap(ci, p0, s - 1, 2),
                                                     start=True, stop=True,
                                                     tile_position=(p0, 0))
                        for h, _ in hs:
                            u = 0 if h == pair[0][0] else 1
                            nc.vector.tensor_add(
                                simS[:, u * SIMW + cs:u * SIMW + cs + csz],
                                simPs[h][:, 0:csz],
                                biasC[ci][:, 0:csz],
                            )

                if pj == 0:
                    # head-serial at the very start: h0's sim completes (and
                    # ACT starts) ~3 µs earlier
                    emit_mm1([pair[0]])
                    emit_mm1([pair[1]])
                else:
                    emit_mm1(pair)
                # V DMAs issued after the K/Q/bias stream (mm2 needs them
                # only ~15 µs later; keeps them off the critical early DMAs)
                Vhs = {}
                for h, p0 in pair:
                    Vh = v_pool.tile([128, NSLOT * 33], bf16, tag="vh",
                                     name=f"vh{h}")
                    nc.sync.dma_start(out=Vh[:, :], in_=vv[h])
                    Vhs[h] = Vh
                # softcap + exp, skipping the 128 filler cols at each end
                # (contents may be garbage). First/last pairs run per-head
                # (last head per-half) ops to shorten the pipeline
                # fill/drain; middle pairs run one wide op per pass.
                pS = p_pool.tile([128, 2 * SIMW], bf16, tag="pS")
                if pj == 0:
                    spans = [(128, SIMW - 128), (SIMW + 128, 2 * SIMW - 128)]
                elif pj == 7:
                    spans = [(128, SIMW - 128),
                             (SIMW + 128, SIMW + 2304),
                             (SIMW + 2304, 2 * SIMW - 128)]
                else:
                    spans = [(128, 2 * SIMW - 128)]
                if pj == 0:
                    # tanh both heads first: the second tanh hides the
                    # same-tile write->read turnaround before the first exp
                    for lo, hi in spans:
                        nc.scalar.activation(simS[:, lo:hi], simS[:, lo:hi],
                                             mybir.ActivationFunctionType.Tanh,
                                             scale=1.0 / SOFTCLAMP)
                    for lo, hi in spans:
                        nc.scalar.activation(pS[:, lo:hi], simS[:, lo:hi],
                                             mybir.ActivationFunctionType.Exp,
                                             scale=SOFTCLAMP)
                else:
                    for lo, hi in spans:
                        nc.scalar.activation(simS[:, lo:hi], simS[:, lo:hi],
                                             mybir.ActivationFunctionType.Tanh,
                                             scale=1.0 / SOFTCLAMP)
                        nc.scalar.activation(pS[:, lo:hi], simS[:, lo:hi],
                                             mybir.ActivationFunctionType.Exp,
                                             scale=SOFTCLAMP)
                # mm2: out (128 q, 33) per task, 8 tasks per PSUM bank
                for u, (h, _) in enumerate(pair):
                    Vh = Vhs[h]
                    base = u * SIMW
                    outW = ow_pool.tile([128, WPC * 33], f32, tag="outW",
                                        name=f"outW{h}")
                    for tb in range(2):
                        otP = out_ps.tile([128, 8 * 33], f32, tag="otP",
                                          name=f"otP{h}_{tb}")
                        for uu in range(8):
                            t = 8 * tb + uu
                            # prev: slot t keys, q_t = second half of slot t block
                            nc.tensor.matmul(
                                otP[:, uu * 33:(uu + 1) * 33],
                                lhsT=pS[:, base + t * 256 + 128:base + t * 256 + 256],
                                rhs=Vh[:, t * 33:(t + 1) * 33],
                                start=True, stop=False)
                            # cur: slot t+1 keys, q_t = first half of slot t+1 block
                            nc.tensor.matmul(
                                otP[:, uu * 33:(uu + 1) * 33],
                                lhsT=pS[:, base + (t + 1) * 256:base + (t + 1) * 256 + 128],
                                rhs=Vh[:, (t + 1) * 33:(t + 2) * 33],
                                start=False, stop=True)
                        nc.vector.tensor_copy(outW[:, tb * 264:(tb + 1) * 264], otP[:, :])
                        nc.sync.dma_start(out=o[h][:, tb * 264:(tb + 1) * 264],
                                          in_=outW[:, tb * 264:(tb + 1) * 264])
    nc.compile()
    return nc


def _get_compiled():
    global _COMPILED
    if _COMPILED is None:
        _COMPILED = _build_bass()
    return _COMPILED


def _prep_core(c, qs, ks, vs, ab, mvec):
    """Build per-core input arrays. qs,ks,vs: (H, N, D) (qs pre-scaled)."""
    w0 = c * WPC
    qw = qs.reshape(H, NW, W, D)[:, w0:w0 + WPC]          # (H,16,128,32)
    qTc = np.ascontiguousarray(
        qw.reshape(4, 4, WPC, W, D).transpose(0, 1, 4, 2, 3).reshape(4, 128, WPC * W))

    kw = ks.reshape(H, NW, W, D)
    vw = vs.reshape(H, NW, W, D)
    khalo = np.zeros((H, NSLOT, W, D), np.float32)
    vhalo = np.zeros((H, NSLOT, W, D), np.float32)
    lo = w0 - 1
    src_lo = max(lo, 0)
    dst_lo = src_lo - lo
    khalo[:, dst_lo:] = kw[:, src_lo:w0 + WPC]
    vhalo[:, dst_lo:] = vw[:, src_lo:w0 + WPC]
    kTc = np.ascontiguousarray(
        khalo.reshape(4, 4, NSLOT, W, D).transpose(0, 1, 4, 2, 3).reshape(4, 128, NSLOT * W))
    vvc = np.concatenate([vhalo, np.ones((H, NSLOT, W, 1), np.float32)], axis=3)
    vvc = np.ascontiguousarray(
        vvc.transpose(0, 2, 1, 3).reshape(H, 128, NSLOT * 33))

    # bias, slot-major: slot s block cols = [cur-bias(task s-1) | prev-bias(task s)]
    # both halves use keys of global window w0+s-1; fold key mask (+ structural
    # masking of window -1) as additive penalty.
    bTc = np.zeros((128, NSLOT, 2, W), np.float32)         # (key, slot, half, q)
    for s in range(NSLOT):
        gw = w0 + s - 1
        if s > 0:
            bTc[:, s, 0, :] = ab[gw, :, 128:256].T          # cur role for task s-1
        if s < NSLOT - 1:
            bTc[:, s, 1, :] = ab[gw + 1, :, 0:128].T        # prev role for task s
        if gw < 0:
            pen = np.full((W,), MASK_PEN, np.float32)
        else:
            pen = np.where(mvec[gw * W:(gw + 1) * W], np.float32(0),
                           np.float32(MASK_PEN))
        bTc[:, s, :, :] += pen[:, None, None]
    bTc = np.ascontiguousarray(bTc.reshape(128, SIMW))
    return {"qT": qTc.astype(BF16), "kT": kTc.astype(BF16),
            "vv": vvc.astype(BF16), "bT": bTc.astype(BF16)}


def _run_device(in_maps, trace=False):
    from concourse.bass_utils import run_bass_kernel_spmd
    nc = _get_compiled()
    res = run_bass_kernel_spmd(nc, in_maps, list(range(NCORES)), trace=trace)
    return res


def kernel(q, k, v, mask, attn_bias, memory_kv, _trace=False, _ret_res=False):
    q = np.asarray(q, np.float32)
    k = np.asarray(k, np.float32)
    v = np.asarray(v, np.float32)
    mask = np.asarray(mask)
    attn_bias = np.asarray(attn_bias, np.float32)
    memory_kv = np.asarray(memory_kv, np.float32)

    qs = q[0] * np.float32(SCALE)       # (H, N, D)
    ks, vs = k[0], v[0]
    ab = attn_bias[0]                   # (NW, W, 2W)
    mvec = mask[0].astype(bool)         # (N,)

    in_maps = [_prep_core(c, qs, ks, vs, ab, mvec) for c in range(NCORES)]
    res = _run_device(in_maps, trace=_trace)
    outs = [r["o"] for r in res.results]             # each (H, 128, WPC*33)

    big = np.stack(outs)                              # (8, H, 128, 528)
    # (core, h, q, task, 33) -> (h, core, task, q, 33) -> (h, n, 33)
    arr = big.reshape(NCORES, H, W, WPC, 33).transpose(1, 0, 3, 2, 4)
    arr = arr.reshape(H, N, 33)
    num = arr[..., :D].astype(np.float64)             # (H, N, D)
    z = arr[..., D].astype(np.float64)                # (H, N)

    # memory-slot attention (4 keys, no bias, mask=True) on host
    mk, mv = memory_kv[0], memory_kv[1]               # (H, 4, D)
    sim_m = np.einsum('hnd,hmd->hnm', qs, mk, dtype=np.float64)
    pm = np.exp(SOFTCLAMP * np.tanh(sim_m / SOFTCLAMP))
    num = num + np.einsum('hnm,hmd->hnd', pm, mv.astype(np.float64))
    z = z + pm.sum(-1)

    out = (num / z[..., None]).astype(np.float32)[None]   # (1, H, N, D)
    if _ret_res:
        return out, res
    return out


# revision 28
# speedup vs baseline: 1.0099x; 1.0088x over previous
"""Windowed sparse attention kernel for TRN2 (8 NeuronCores).

Problem: b=1, h=16, n=16384, d=32, window w=128, nw=128 windows.
Each window of 128 queries attends to [4 memory slots | prev window | cur window]
with additive bias, tanh softcap (50), softmax.

Sharding: sequence-parallel over windows. Core c handles windows
[c*16, (c+1)*16) for all 16 heads, with a one-window k/v halo.

Device dataflow (keys on partitions, slot-major, bf16 matmuls):
  For k/v slot s (17 per core), one N=256 bf16 matmul computes
  simT[key_s, (q_{s-1} | q_s)] — slot s's keys against both query windows
  that attend to it. Heads are processed in pairs on distinct 32-row
  PE strips (tile_position) so their K=32 matmuls overlap in the array.
  DVE adds the (pre-arranged, mask-folded) bf16 bias while evacuating
  PSUM in 1024-col chunks into a pair-wide fp32 sim tile. ACT applies
  tanh (in-place) + exp in two 8448-col instructions per pair (exp
  output bf16), skipping the filler columns at both ends. mm2 flips
  orientation: lhsT = p-slice (keys x queries, bf16), rhs = v~
  (keys x 33, bf16) -> out (128 q, 33) per task, where v~'s ones column
  makes col 32 the softmax denominator Z. Host combines the 4-slot
  memory attention (1.5% of keys) and normalizes.
"""

import numpy as np
import ml_dtypes

B, H, N, D = 1, 16, 16384, 32
W = 128                 # window size
NW = N // W             # 128 windows
NCORES = 8
WPC = NW // NCORES      # 16 windows (tasks) per core
NSLOT = WPC + 1         # 17 k/v slots (halo)
SOFTCLAMP = 50.0
SCALE = D ** -0.5
MASK_PEN = -30000.0
SIMW = NSLOT * 256      # 4352 wide-tile cols (slot-major, 256 per slot)
CHUNK = 1536            # PSUM evacuation chunk (3 banks, 6 slots)

BF16 = ml_dtypes.bfloat16

_COMPILED = None


def _build_bass():
    import concourse.bacc as bacc
    import concourse.tile as tile
    from concourse import mybir
    from contextlib import ExitStack

    f32 = mybir.dt.float32
    bf16 = mybir.dt.bfloat16
    nc = bacc.Bacc()

    qT = nc.declare_dram_parameter("qT", [4, 128, WPC * W], bf16, isOutput=False)
    kT = nc.declare_dram_parameter("kT", [4, 128, NSLOT * W], bf16, isOutput=False)
    vv = nc.declare_dram_parameter("vv", [H, 128, NSLOT * 33], bf16, isOutput=False)
    bT = nc.declare_dram_parameter("bT", [128, SIMW], bf16, isOutput=False)
    o = nc.declare_dram_parameter("o", [H, 128, WPC * 33], f32, isOutput=True)

    # chunk layout: (col_start, ncols, slots)
    chunks = [(0, 1536, list(range(0, 6))),
              (1536, 1536, list(range(6, 12))),
              (3072, 1280, list(range(12, 17)))]

    with ExitStack() as ctx:
        tc = ctx.enter_context(tile.TileContext(nc))
        singles = ctx.enter_context(tc.tile_pool(name="singles", bufs=1))
        qk_pool = ctx.enter_context(tc.tile_pool(name="qk", bufs=2))
        v_pool = ctx.enter_context(tc.tile_pool(name="v", bufs=4))
        sim_pool = ctx.enter_context(tc.tile_pool(name="sims", bufs=3))
        p_pool = ctx.enter_context(tc.tile_pool(name="ps", bufs=2))
        ow_pool = ctx.enter_context(tc.tile_pool(name="ow", bufs=2))
        sim_ps = ctx.enter_context(tc.tile_pool(name="simps", bufs=2, space="PSUM"))
        out_ps = ctx.enter_context(tc.tile_pool(name="outps", bufs=2, space="PSUM"))

        # per-chunk bias tiles: the first DVE add only waits on its own slice
        biasC = [None, None, None]

        def emit_bias(ci):
            cs, csz, _ = chunks[ci]
            bt = singles.tile([128, csz], bf16, name=f"bias{ci}")
            eng = nc.gpsimd if ci == 0 else nc.sync
            eng.dma_start(out=bt[:, :], in_=bT[:, cs:cs + csz])
            biasC[ci] = bt

        # K/Q tiles per PSUM chunk: chunk ci's matmuls gate only on slice ci.
        # K slices by slot: [0:6), [6:12), [12:17); Q slices by window with
        # one-window overlap: [0:6), [5:12), [11:16).
        KSL = [(0, 6 * W), (6 * W, 12 * W), (12 * W, NSLOT * W)]
        QSL = [(0, 6 * W), (5 * W, 12 * W), (11 * W, WPC * W)]
        for g in range(4):
            Ks, Qs = [], []
            for ci in range(3):
                # the idle GpSimd engine's preamble ends earliest — its SWDGE
                # issues the critical first transfers ~2 µs sooner than HWDGE
                dma = nc.gpsimd if (g == 0 and ci == 0) else nc.sync
                ks0, ks1 = KSL[ci]
                Kc = qk_pool.tile([128, ks1 - ks0], bf16, tag=f"k{ci}",
                                  name=f"k{ci}g{g}")
                dma.dma_start(out=Kc[:, :], in_=kT[g][:, ks0:ks1])
                qs0, qs1 = QSL[ci]
                Qc = qk_pool.tile([128, qs1 - qs0], bf16, tag=f"q{ci}",
                                  name=f"q{ci}g{g}")
                dma.dma_start(out=Qc[:, :], in_=qT[g][:, qs0:qs1])
                Ks.append(Kc)
                Qs.append(Qc)
                if g == 0:
                    emit_bias(ci)       # need-order: K_ci, Q_ci, bias_ci

            def k_ap(ci, p0, s):
                lo = s * W - KSL[ci][0]
                return Ks[ci][p0:p0 + 32, lo:lo + W]

            def q_ap(ci, p0, w, nw):    # query windows w .. w+nw-1
                lo = w * W - QSL[ci][0]
                return Qs[ci][p0:p0 + 32, lo:lo + nw * W]

            for j in range(2):      # head pairs within group, strips (64j, 64j+32)
                pj = 2 * g + j      # pair index 0..7
                pair = [(4 * g + 2 * j + u, 32 * (2 * j + u)) for u in range(2)]
                simS = sim_pool.tile([128, 2 * SIMW], f32, tag="simS")

                def emit_mm1(hs):
                    # mm1 + bias-add for the heads in hs, chunk by chunk,
                    # interleaved across their row strips
                    for ci, (cs, csz, slots) in enumerate(chunks):
                        simPs = {h: sim_ps.tile([128, CHUNK], f32, tag="simP",
                                                name=f"simP{h}c{ci}")
                                 for h, _ in hs}
                        for s in slots:
                            off = s * 256 - cs
                            for h, p0 in hs:
                                simP = simPs[h]
                                lhsT = k_ap(ci, p0, s)
                                if s == 0:
                                    # cols [0:128] (task -1) are filler: never
                                    # computed, skipped by ACT, unread by mm2
                                    nc.tensor.matmul(simP[:, 128:256], lhsT=lhsT,
                                                     rhs=q_ap(ci, p0, 0, 1),
                                                     start=True, stop=True,
                                                     tile_position=(p0, 0))
                                elif s == NSLOT - 1:
                                    nc.tensor.matmul(simP[:, off:off + 128], lhsT=lhsT,
                                                     rhs=q_ap(ci, p0, s - 1, 1),
                                                     start=True, stop=True,
                                                     tile_position=(p0, 0))
                                else:
                                    nc.tensor.matmul(simP[:, off:off + 256], lhsT=lhsT,
                                                     rhs=q_ap(ci, p0, s - 1, 2),
                                                     start=True, stop=True,
                                                     tile_position=(p0, 0))
                        # skip the filler cols (task -1 / slot-16 second half):
                        # ACT skips them and mm2 never reads them
                        lo = 128 if ci == 0 else 0
                        hi = csz - 128 if ci == 2 else csz
                        for h, _ in hs:
                            u = 0 if h == pair[0][0] else 1
                            nc.vector.tensor_add(
                                simS[:, u * SIMW + cs + lo:u * SIMW + cs + hi],
                                simPs[h][:, lo:hi],
                                biasC[ci][:, lo:hi],
                            )

                if pj == 0:
                    # head-serial at the very start: h0's sim completes (and
                    # ACT starts) ~3 µs earlier
                    emit_mm1([pair[0]])
                    emit_mm1([pair[1]])
                else:
                    emit_mm1(pair)
                # V DMAs issued after the K/Q/bias stream (mm2 needs them
                # only ~15 µs later; keeps them off the critical early DMAs)
                Vhs = {}
                for h, p0 in pair:
                    Vh = v_pool.tile([128, NSLOT * 33], bf16, tag="vh",
                                     name=f"vh{h}")
                    nc.sync.dma_start(out=Vh[:, :], in_=vv[h])
                    Vhs[h] = Vh
                # softcap + exp, skipping the 128 filler cols at each end
                # (contents may be garbage). First/last pairs run per-head
                # (last head per-half) ops to shorten the pipeline
                # fill/drain; middle pairs run one wide op per pass.
                pS = p_pool.tile([128, 2 * SIMW], bf16, tag="pS")
                if pj == 0:
                    spans = [(128, SIMW - 128), (SIMW + 128, 2 * SIMW - 128)]
                elif pj == 7:
                    spans = [(128, SIMW - 128),
                             (SIMW + 128, SIMW + 2304),
                             (SIMW + 2304, 2 * SIMW - 128)]
                else:
                    spans = [(128, 2 * SIMW - 128)]
                if pj == 0:
                    # tanh both heads first: the second tanh hides the
                    # same-tile write->read turnaround before the first exp
                    for lo, hi in spans:
                        nc.scalar.activation(simS[:, lo:hi], simS[:, lo:hi],
                                             mybir.ActivationFunctionType.Tanh,
                                             scale=1.0 / SOFTCLAMP)
                    for lo, hi in spans:
                        nc.scalar.activation(pS[:, lo:hi], simS[:, lo:hi],
                                             mybir.ActivationFunctionType.Exp,
                                             scale=SOFTCLAMP)
                else:
                    for lo, hi in spans:
                        nc.scalar.activation(simS[:, lo:hi], simS[:, lo:hi],
                                             mybir.ActivationFunctionType.Tanh,
                                             scale=1.0 / SOFTCLAMP)
                        nc.scalar.activation(pS[:, lo:hi], simS[:, lo:hi],
                                             mybir.ActivationFunctionType.Exp,
                                             scale=SOFTCLAMP)
                # mm2: out (128 q, 33) per task, 8 tasks per PSUM bank
                for u, (h, _) in enumerate(pair):
                    Vh = Vhs[h]
                    base = u * SIMW
                    outW = ow_pool.tile([128, WPC * 33], f32, tag="outW",
                                        name=f"outW{h}")
                    for tb in range(2):
                        otP = out_ps.tile([128, 8 * 33], f32, tag="otP",
                                          name=f"otP{h}_{tb}")
                        for uu in range(8):
                            t = 8 * tb + uu
                            # prev: slot t keys, q_t = second half of slot t block
                            nc.tensor.matmul(
                                otP[:, uu * 33:(uu + 1) * 33],
                                lhsT=pS[:, base + t * 256 + 128:base + t * 256 + 256],
                                rhs=Vh[:, t * 33:(t + 1) * 33],
                                start=True, stop=False)
                            # cur: slot t+1 keys, q_t = first half of slot t+1 block
                            nc.tensor.matmul(
                                otP[:, uu * 33:(uu + 1) * 33],
                                lhsT=pS[:, base + (t + 1) * 256:base + (t + 1) * 256 + 128],
                                rhs=Vh[:, (t + 1) * 33:(t + 2) * 33],
                                start=False, stop=True)
                        nc.vector.tensor_copy(outW[:, tb * 264:(tb + 1) * 264], otP[:, :])
                        nc.sync.dma_start(out=o[h][:, tb * 264:(tb + 1) * 264],
                                          in_=outW[:, tb * 264:(tb + 1) * 264])
    nc.compile()
    return nc


def _get_compiled():
    global _COMPILED
    if _COMPILED is None:
        _COMPILED = _build_bass()
    return _COMPILED


def _prep_core(c, qs, ks, vs, ab, mvec):
    """Build per-core input arrays. qs,ks,vs: (H, N, D) (qs pre-scaled)."""
    w0 = c * WPC
    qw = qs.reshape(H, NW, W, D)[:, w0:w0 + WPC]          # (H,16,128,32)
    qTc = np.ascontiguousarray(
        qw.reshape(4, 4, WPC, W, D).transpose(0, 1, 4, 2, 3).reshape(4, 128, WPC * W))

    kw = ks.reshape(H, NW, W, D)
    vw = vs.reshape(H, NW, W, D)
    khalo = np.zeros((H, NSLOT, W, D), np.float32)
    vhalo = np.zeros((H, NSLOT, W, D), np.float32)
    lo = w0 - 1
    src_lo = max(lo, 0)
    dst_lo = src_lo - lo
    khalo[:, dst_lo:] = kw[:, src_lo:w0 + WPC]
    vhalo[:, dst_lo:] = vw[:, src_lo:w0 + WPC]
    kTc = np.ascontiguousarray(
        khalo.reshape(4, 4, NSLOT, W, D).transpose(0, 1, 4, 2, 3).reshape(4, 128, NSLOT * W))
    vvc = np.concatenate([vhalo, np.ones((H, NSLOT, W, 1), np.float32)], axis=3)
    vvc = np.ascontiguousarray(
        vvc.transpose(0, 2, 1, 3).reshape(H, 128, NSLOT * 33))

    # bias, slot-major: slot s block cols = [cur-bias(task s-1) | prev-bias(task s)]
    # both halves use keys of global window w0+s-1; fold key mask (+ structural
    # masking of window -1) as additive penalty.
    bTc = np.zeros((128, NSLOT, 2, W), np.float32)         # (key, slot, half, q)
    for s in range(NSLOT):
        gw = w0 + s - 1
        if s > 0:
            bTc[:, s, 0, :] = ab[gw, :, 128:256].T          # cur role for task s-1
        if s < NSLOT - 1:
            bTc[:, s, 1, :] = ab[gw + 1, :, 0:128].T        # prev role for task s
        if gw < 0:
            pen = np.full((W,), MASK_PEN, np.float32)
        else:
            pen = np.where(mvec[gw * W:(gw + 1) * W], np.float32(0),
                           np.float32(MASK_PEN))
        bTc[:, s, :, :] += pen[:, None, None]
    bTc = np.ascontiguousarray(bTc.reshape(128, SIMW))
    return {"qT": qTc.astype(BF16), "kT": kTc.astype(BF16),
            "vv": vvc.astype(BF16), "bT": bTc.astype(BF16)}


def _run_device(in_maps, trace=False):
    from concourse.bass_utils import run_bass_kernel_spmd
    nc = _get_compiled()
    res = run_bass_kernel_spmd(nc, in_maps, list(range(NCORES)), trace=trace)
    return res


def kernel(q, k, v, mask, attn_bias, memory_kv, _trace=False, _ret_res=False):
    q = np.asarray(q, np.float32)
    k = np.asarray(k, np.float32)
    v = np.asarray(v, np.float32)
    mask = np.asarray(mask)
    attn_bias = np.asarray(attn_bias, np.float32)
    memory_kv = np.asarray(memory_kv, np.float32)

    qs = q[0] * np.float32(SCALE)       # (H, N, D)
    ks, vs = k[0], v[0]
    ab = attn_bias[0]                   # (NW, W, 2W)
    mvec = mask[0].astype(bool)         # (N,)

    in_maps = [_prep_core(c, qs, ks, vs, ab, mvec) for c in range(NCORES)]
    res = _run_device(in_maps, trace=_trace)
    outs = [r["o"] for r in res.results]             # each (H, 128, WPC*33)

    big = np.stack(outs)                              # (8, H, 128, 528)
    # (core, h, q, task, 33) -> (h, core, task, q, 33) -> (h, n, 33)
    arr = big.reshape(NCORES, H, W, WPC, 33).transpose(1, 0, 3, 2, 4)
    arr = arr.reshape(H, N, 33)
    num = arr[..., :D].astype(np.float64)             # (H, N, D)
    z = arr[..., D].astype(np.float64)                # (H, N)

    # memory-slot attention (4 keys, no bias, mask=True) on host
    mk, mv = memory_kv[0], memory_kv[1]               # (H, 4, D)
    sim_m = np.einsum('hnd,hmd->hnm', qs, mk, dtype=np.float64)
    pm = np.exp(SOFTCLAMP * np.tanh(sim_m / SOFTCLAMP))
    num = num + np.einsum('hnm,hmd->hnd', pm, mv.astype(np.float64))
    z = z + pm.sum(-1)

    out = (num / z[..., None]).astype(np.float32)[None]   # (1, H, N, D)
    if _ret_res:
        return out, res
    return out
